# revision 29
# baseline (speedup 1.0000x reference)
"""Trainium2 Bass kernel for MultiHeadSelfAttention (GroupNorm + QKV + attention + proj + residual).

Problem shape (hardcoded): x [8, 512, 32, 32] fp32, 8 heads, 32 groups.
Sharding: data-parallel over batch B=8 across the 8 NeuronCores (one batch per core).

Per-core pipeline (T = 1024 positions, C = 512 channels, ch = 64 per head):
  1. GroupNorm(32) over [C, T] from a bf16 copy of x: per-channel bn_stats,
     group-combine via tiny PE matmuls, rsqrt via ACT ln/exp; the affine
     writes h in fp8e4m3 (split across DVE and Pool engines).
  2. qkv: fp8 DoubleRow matmuls (0.5 cyc/row, 2 stacked c-chunks per
     instruction). q,k land in bf16 tiles via ACT copies (bias added on the
     PE by a K=1 fp8 rider matmul). v is computed transposed (vT [T, 64*8])
     with a constant-8.0 rider column per head for the softmax denominators.
  3. Per head: QK^T on PE in bf16 (contraction ch=64), exp split between the
     ACT engine (native Exp -> fp8 out) and DVE (Schraudolph bit-trick:
     bits(fp8e4m3) = trunc(8*log2e*x + 56.04) written as int8 and bitcast).
     AV pair-packs two heads into one 128-row psum (fp8 DoubleRow); softmax
     denominators come from a block-diagonal 8.0-ones DoubleRow matmul.
     Normalization: reciprocal on DVE, partition-broadcast on Pool,
     one fp8 multiply per head-pair on DVE.
  4. proj: fp8 DoubleRow + bias rider + 64*identity x-rider on PE; the final
     residual is a single ACT copy with scale 1/64 to bf16 output.

All fp8 scale factors (weights x64, v x64, a x8, proj x8) are folded into the
host-preprocessed weights, the exp constants, and the rider values.
"""

import ml_dtypes
import numpy as np

import concourse.bacc as bacc
import concourse.tile as tile
import concourse.mybir as mybir
from concourse import library_config
from concourse.bass_utils import run_bass_kernel_spmd

B, C, HS, WS = 8, 512, 32, 32
T = HS * WS            # 1024
H = 8                  # heads
CH = C // H            # 64
G = 32                 # groups
CPG = C // G           # 16 channels per group
EPS = 1e-5
NCHUNK = C // 128      # 4 channel chunks
NT = T // 128          # 8 sequence tiles
F32 = mybir.dt.float32
F32R = mybir.dt.float32r
BF16 = mybir.dt.bfloat16
FP8 = mybir.dt.float8e4
I8 = mybir.dt.int8
DR = mybir.MatmulPerfMode.DoubleRow

SW = 64.0              # fp8 scale on qkv/v weights
SA = 8.0               # fp8 scale on a (rider value = SW/SA = 8)
SP = 8.0               # fp8 scale on proj weights -> psum = SA*SP = 64x
ALPHA = 1.0 / (SW * SW)       # logit descale for exp
LOG2E = 1.4426950408889634
EXP_A = 8.0 * LOG2E * ALPHA   # DVE bit-trick multiplier
EXP_B = 8.0 * 7.0 + 0.04      # DVE bit-trick bias (trunc-to-int8 calibrated)

_CACHE = {}


def _orig_row(kind, h, i):
    off = {"q": 0, "k": CH, "v": 2 * CH}[kind]
    return 192 * h + off + i


def _to_fp8(a):
    return np.asarray(a, np.float32).astype(ml_dtypes.float8_e4m3)


def _host_weights(gn_w, gn_b, qkv_w, qkv_b, proj_w, proj_b):
    f = np.float64
    qkv_w = np.asarray(qkv_w, f)
    qkv_b = np.asarray(qkv_b, f)
    proj_w = np.asarray(proj_w, f)
    proj_b = np.asarray(proj_b, f)
    scale2 = 1.0 / np.sqrt(CH)  # ch**-0.25 on both q and k -> fold into k

    # qk row order: m-tile 2p = [k_h(2p) | k_h(2p+1)], m-tile 2p+1 = [q..]
    rows = np.zeros(2 * C, dtype=np.int64)
    colscale = np.ones(2 * C, dtype=f)
    for p in range(H // 2):
        for slot in range(2):
            h = 2 * p + slot
            for i in range(CH):
                col_k = (2 * p) * 128 + slot * CH + i
                rows[col_k] = _orig_row("k", h, i)
                colscale[col_k] = scale2
                col_q = (2 * p + 1) * 128 + slot * CH + i
                rows[col_q] = _orig_row("q", h, i)
    wqk = qkv_w[rows, :] * colscale[:, None] * SW   # [1024 rows, 512 c]
    # layout [c(128), m(8), pair(2), j(2), mcol(128)]
    wqk_l = np.zeros((128, 8, 2, 2, 128), dtype=f)
    for m in range(8):
        for i in range(2):
            for j in range(2):
                cg = (2 * i + j) * 128
                wqk_l[:, m, i, j, :] = wqk[m * 128:(m + 1) * 128,
                                          cg:cg + 128].T
    bqk = (qkv_b[rows] * colscale * SW)             # [1024]
    bqk_l = np.zeros((1, 8, 2, 128), dtype=f)
    bqk_l[0, :, 0, :] = bqk.reshape(8, 128)

    vrows = np.array([_orig_row("v", h, i) for h in range(H)
                      for i in range(CH)])
    wv = qkv_w[vrows, :] * SW                       # [512 cv, 512 c]
    wv_l = np.zeros((128, 2, 2, C), dtype=f)        # [c, pair, j, cv]
    for i in range(2):
        for j in range(2):
            cg = (2 * i + j) * 128
            wv_l[:, i, j, :] = wv[:, cg:cg + 128].T
    bv = qkv_b[vrows]                               # [512] folded into bproj

    wproj = proj_w * SP                             # [512 o, 512 c]
    wproj_l = np.zeros((128, 4, 2, 2, 128), dtype=f)  # [c, m, pair, j, o]
    for m in range(4):
        for i in range(2):
            for j in range(2):
                cg = (2 * i + j) * 128
                wproj_l[:, m, i, j, :] = wproj[m * 128:(m + 1) * 128,
                                               cg:cg + 128].T
    bproj_eff = (proj_b + proj_w @ bv) * SA * SP    # v-bias folded, x64
    bproj_l = np.zeros((1, 4, 2, 128), dtype=f)
    bproj_l[0, :, 0, :] = bproj_eff.reshape(4, 128)

    gnw = np.asarray(gn_w, f).reshape(NCHUNK, 128).T.copy()
    gnb = np.asarray(gn_b, f).reshape(NCHUNK, 128).T.copy()

    g_all = np.zeros((128, 128), dtype=np.float32)
    gt_all = np.zeros((32, 512), dtype=np.float32)
    for k in range(NCHUNK):
        for u in range(128):
            g = 8 * k + u // CPG
            g_all[u, 32 * k + g] = 1.0 / CPG
            gt_all[g, 128 * k + u] = 1.0

    ident64 = (64.0 * np.eye(128, dtype=np.float32)).astype(ml_dtypes.bfloat16)
    return {
        "wqk": _to_fp8(wqk_l), "bqk": _to_fp8(bqk_l),
        "wv": _to_fp8(wv_l),
        "wproj": _to_fp8(wproj_l), "bproj": _to_fp8(bproj_l),
        "gnw": gnw.astype(np.float32), "gnb": gnb.astype(np.float32),
        "g_all": g_all, "gt_all": gt_all, "ident64": ident64,
    }


def _build_program(n_reps=1, act_share=22, debug=False):
    """act_share: of the 8 exp tiles per head, how many go to ACT (rest DVE)."""
    nc = bacc.Bacc("TRN2", target_bir_lowering=False, debug=False,
                   num_devices=8)
    dt_in = [
        ("x", [C // 128, 128, T], BF16),
        ("wqk", [128, 8, 2, 2, 128], FP8), ("bqk", [1, 8, 2, 128], FP8),
        ("wv", [128, 2, 2, C], FP8),
        ("wproj", [128, 4, 2, 2, 128], FP8), ("bproj", [1, 4, 2, 128], FP8),
        ("gnw", [128, NCHUNK], F32), ("gnb", [128, NCHUNK], F32),
        ("g_all", [128, 128], F32R), ("gt_all", [32, 512], F32R),
        ("ident64", [128, 128], BF16),
    ]
    d = {name: nc.dram_tensor(name, shape, dt, kind="ExternalInput").ap()
         for name, shape, dt in dt_in}
    out_d = nc.dram_tensor("out", [NCHUNK, 128, T], BF16,
                           kind="ExternalOutput").ap()
    if debug:
        dbg_qk = nc.dram_tensor("dbg_qk", [2, 128, T], BF16,
                                kind="ExternalOutput").ap()
        dbg_ew = nc.dram_tensor("dbg_ew", [128, NT, T], FP8,
                                kind="ExternalOutput").ap()
        dbg_pa = nc.dram_tensor("dbg_pa", [H, 128, T], F32,
                                kind="ExternalOutput").ap()
        dbg_h4 = nc.dram_tensor("dbg_h4", [128, NCHUNK, T], FP8,
                                kind="ExternalOutput").ap()
        dbg_vt = nc.dram_tensor("dbg_vt", [128, NT, H, 128], FP8,
                                kind="ExternalOutput").ap()
        dbg_a4 = nc.dram_tensor("dbg_a4", [128, NCHUNK, T], FP8,
                                kind="ExternalOutput").ap()
        dbg_rec = nc.dram_tensor("dbg_rec", [H, T], F32,
                                 kind="ExternalOutput").ap()
        dbg_rb = nc.dram_tensor("dbg_rb", [H, T], F32,
                                kind="ExternalOutput").ap()

    with tile.TileContext(nc) as tc:
        with (
            tc.tile_pool(name="singles", bufs=1) as singles,
            tc.tile_pool(name="small", bufs=10) as small,
            tc.tile_pool(name="recp", bufs=2) as recp,
            tc.tile_pool(name="rbp", bufs=2) as rbp,
            tc.tile_pool(name="outp", bufs=4) as outp,
            tc.tile_pool(name="pA", bufs=3, space="PSUM") as pA,
            tc.tile_pool(name="pPair", bufs=1, space="PSUM") as pPair,
        ):
            nc.gpsimd.load_library(library_config.attn)

            # ---- input DMAs: x on sync queue, weights on gpsimd queue ----
            x_sb = []
            for k in range(NCHUNK):
                t_ = singles.tile([128, T], BF16, tag=f"x{k}", name=f"x{k}")
                nc.sync.dma_start(t_[:], d["x"][k])
                x_sb.append(t_)
            g_sb = singles.tile([128, 128], F32R, tag="g_all", name="g_sb")
            nc.sync.dma_start(g_sb[:], d["g_all"][:])
            gt_sb = singles.tile([32, 512], F32R, tag="gt_all", name="gt_sb")
            nc.sync.dma_start(gt_sb[:], d["gt_all"][:])
            gnw_sb = singles.tile([128, NCHUNK], F32, tag="gnw", name="gnw_sb")
            nc.sync.dma_start(gnw_sb[:], d["gnw"][:])
            gnb_sb = singles.tile([128, NCHUNK], F32, tag="gnb", name="gnb_sb")
            nc.sync.dma_start(gnb_sb[:], d["gnb"][:])
            wqk_sb = singles.tile([128, 8, 2, 2, 128], FP8, tag="wqk",
                                  name="wqk_sb")
            nc.sync.dma_start(wqk_sb[:], d["wqk"][:])
            bqk_sb = singles.tile([1, 8, 2, 128], FP8, tag="bqk",
                                  name="bqk_sb")
            nc.sync.dma_start(bqk_sb[:], d["bqk"][:])
            wv_sb = singles.tile([128, 2, 2, C], FP8, tag="wv", name="wv_sb")
            nc.sync.dma_start(wv_sb[:], d["wv"][:])
            wproj_sb = singles.tile([128, 4, 2, 2, 128], FP8, tag="wproj",
                                    name="wproj_sb")
            nc.sync.dma_start(wproj_sb[:], d["wproj"][:])
            bproj_sb = singles.tile([1, 4, 2, 128], FP8, tag="bproj",
                                    name="bproj_sb")
            nc.sync.dma_start(bproj_sb[:], d["bproj"][:])
            id64_sb = singles.tile([128, 128], BF16, tag="ident64",
                                   name="id64_sb")
            nc.sync.dma_start(id64_sb[:], d["ident64"][:])

            eps_t = singles.tile([32, 1], F32, tag="eps", name="eps_t")
            nc.vector.memset(eps_t[:], EPS)
            # DoubleRow rider rhs: ones [1, 2, 256] fp8
            onesdr = singles.tile([1, 2, 256], FP8, tag="onesdr",
                                  name="onesdr")
            nc.vector.memset(onesdr[:], 1.0)

            for rep in range(n_reps):
                sfx = f"r{rep}"
                # ================= GroupNorm =================
                stats_list = []
                for k in range(NCHUNK):
                    st6 = small.tile([128, 2, 6], F32, tag="small",
                                     name=f"st6{k}{sfx}")
                    nc.vector.bn_stats(st6[:, 0, :], x_sb[k][:, 0:512])
                    nc.vector.bn_stats(st6[:, 1, :], x_sb[k][:, 512:1024])
                    mv = small.tile([128, 2], F32, tag="small",
                                    name=f"mv{k}{sfx}")
                    nc.vector.bn_aggr(mv[:], st6[:])
                    m2 = small.tile([128, 1], F32, tag="small",
                                    name=f"m2{k}{sfx}")
                    nc.vector.tensor_mul(m2[:], mv[:, 0:1], mv[:, 0:1])
                    stats = small.tile([128, 2], F32R, tag="small",
                                       name=f"stats{k}{sfx}")
                    nc.vector.tensor_copy(stats[:, 0:1], mv[:, 0:1])
                    nc.vector.tensor_add(stats[:, 1:2], mv[:, 1:2], m2[:])
                    stats_list.append(stats)
                psum_gs = pA.tile([128, T], F32, tag="pA", name="psum_gs")
                for k in range(NCHUNK):
                    nc.tensor.matmul(psum_gs[0:32, 0:2],
                                     g_sb[:, 32 * k:32 * (k + 1)],
                                     stats_list[k][:],
                                     start=(k == 0), stop=(k == 3))
                gsb = small.tile([32, 2], F32, tag="small", name=f"gsb{sfx}")
                nc.vector.tensor_copy(gsb[:], psum_gs[0:32, 0:2])
                mu2 = small.tile([32, 1], F32, tag="small", name=f"mu2{sfx}")
                nc.vector.tensor_mul(mu2[:], gsb[:, 0:1], gsb[:, 0:1])
                varg = small.tile([32, 1], F32, tag="small", name=f"varg{sfx}")
                nc.vector.tensor_sub(varg[:], gsb[:, 1:2], mu2[:])
                vpe = small.tile([32, 1], F32, tag="small", name=f"vpe{sfx}")
                nc.vector.tensor_scalar(out=vpe[:], in0=varg[:],
                                        scalar1=EPS, scalar2=None,
                                        op0=mybir.AluOpType.add)
                y0 = small.tile([32, 1], F32, tag="small", name=f"y0{sfx}")
                nc.vector.tensor_scalar(out=y0[:].bitcast(mybir.dt.int32),
                                        in0=vpe[:].bitcast(mybir.dt.int32),
                                        scalar1=-0.5, scalar2=1597463007.0,
                                        op0=mybir.AluOpType.mult,
                                        op1=mybir.AluOpType.add)
                t1r = small.tile([32, 1], F32, tag="small", name=f"t1r{sfx}")
                nc.vector.tensor_mul(t1r[:], vpe[:], y0[:])
                t2r = small.tile([32, 1], F32, tag="small", name=f"t2r{sfx}")
                nc.vector.tensor_mul(t2r[:], t1r[:], y0[:])
                t3r = small.tile([32, 1], F32, tag="small", name=f"t3r{sfx}")
                nc.vector.tensor_scalar(out=t3r[:], in0=t2r[:],
                                        scalar1=-0.5, scalar2=1.5,
                                        op0=mybir.AluOpType.mult,
                                        op1=mybir.AluOpType.add)
                rstd = small.tile([32, 1], F32, tag="small", name=f"rstd{sfx}")
                nc.vector.tensor_mul(rstd[:], y0[:], t3r[:])
                grp = small.tile([32, 2], F32R, tag="small", name=f"grp{sfx}")
                nc.vector.tensor_copy(grp[:, 0:1], gsb[:, 0:1])
                nc.vector.tensor_copy(grp[:, 1:2], rstd[:])
                h4 = singles.tile([128, NCHUNK, T], FP8, tag="h4",
                                  name=f"h4{sfx}")
                # all 4 chunks' (mu, rstd) in one psum [128, 2, 4]; batched
                # s/b computation with strided slices
                psum_pc = pA.tile([128, T], F32, tag="pA", name="psum_pc")
                for k in range(NCHUNK):
                    nc.tensor.matmul(psum_pc[:, 2 * k:2 * k + 2],
                                     gt_sb[:, 128 * k:128 * (k + 1)],
                                     grp[:], start=(k == 0),
                                     stop=(k == 3), skip_group_check=True)
                pcv = psum_pc[:, 0:8].rearrange("p (k two) -> p k two", two=2)
                s_c = small.tile([128, 4], F32, tag="small", name=f"sc{sfx}")
                nc.vector.tensor_mul(s_c[:], pcv[:, :, 1], gnw_sb[:])
                t1 = small.tile([128, 4], F32, tag="small", name=f"t1{sfx}")
                nc.vector.tensor_mul(t1[:], pcv[:, :, 0], s_c[:])
                b_c = small.tile([128, 4], F32, tag="small", name=f"bc{sfx}")
                nc.vector.tensor_sub(b_c[:], gnb_sb[:], t1[:])
                for k in range(NCHUNK):
                    nc.vector.tensor_scalar(
                        out=h4[:, k, :], in0=x_sb[k][:],
                        scalar1=s_c[:, k:k + 1], scalar2=b_c[:, k:k + 1],
                        op0=mybir.AluOpType.mult,
                        op1=mybir.AluOpType.add)

                # ============ vT tile + rider column ============
                # vt cols: [0]=8.0 rider (softmax denom), [1:32]=0 pad so
                # the v block starts 32-aligned (engine partition rule) and
                # reciprocal_approx_fast reads pa at partition 0 (custom DVE
                # ops mishandle AP offsets), [32:96]=v. M=96 even and pair
                # stride 8*96=768 (16-multiple) per dual-fp8 ldweights rules.
                vt = singles.tile([128, NT, H, 128], FP8, tag="vt",
                                  name=f"vt{sfx}")
                nc.gpsimd.memset(vt[:, :, :, 0:1], SA)
                nc.gpsimd.memset(vt[:, :, :, 1:64], 0.0)

                def emit_v_tile(st):
                    pv = pPair.tile([128, 512], F32, tag="pPair", name="pv")
                    for i in range(2):
                        for cv in range(2):
                            nc.tensor.matmul(
                                pv[:, 256 * cv:256 * (cv + 1)],
                                h4[:, 2 * i:2 * i + 2,
                                   128 * st:128 * (st + 1)],
                                wv_sb[:, i, :, 256 * cv:256 * (cv + 1)],
                                start=(i == 0 and cv == 0),
                                stop=(i == 1), perf_mode=DR,
                                skip_group_check=True)
                    nc.vector.tensor_copy(
                        vt[:, st, :, 64:64 + CH],
                        pv[:].rearrange("p (h c) -> p h c", h=H))

                # ============ qkv m-tile -> bf16 qk tile ============
                qk = {}

                def emit_qk_tile(m):
                    pq = pA.tile([128, T], F32, tag="pA", name="pq")
                    for i in range(2):
                        for ch4 in range(4):
                            sl = slice(256 * ch4, 256 * (ch4 + 1))
                            nc.tensor.matmul(
                                pq[:, sl], wqk_sb[:, m, i],
                                h4[:, 2 * i:2 * i + 2, sl],
                                start=(i == 0 and ch4 % 2 == 0),
                                stop=False, perf_mode=DR,
                                skip_group_check=True)
                    for ch4 in range(4):
                        sl = slice(256 * ch4, 256 * (ch4 + 1))
                        nc.tensor.matmul(
                            pq[:, sl], bqk_sb[:, m], onesdr[:],
                            start=False, stop=True, perf_mode=DR,
                            skip_group_check=True)
                    qt = singles.tile([128, T], BF16, tag=f"qk{m}",
                                      name=f"qk{m}{sfx}")
                    nc.scalar.activation(qt[:], pq[:],
                                         mybir.ActivationFunctionType.Copy)
                    qk[m] = qt

                # PE stream: pair-0 qk tiles upfront; V-tiles paced one per
                # attention step; qk m2-7 paced one per ~3 steps
                for m in range(2):
                    emit_qk_tile(m)

                def v_duty():
                    for st in range(NT):
                        emit_v_tile(st)
                        yield
                        yield
                        yield
                        yield

                def qk_duty():
                    for m in range(2, 8):
                        emit_qk_tile(m)
                        for _ in range(12):
                            yield
                if debug:
                    nc.sync.dma_start(dbg_qk[0], qk[0][:])
                    nc.sync.dma_start(dbg_qk[1], qk[1][:])
                    nc.sync.dma_start(dbg_h4[:], h4[:])
                    nc.sync.dma_start(dbg_vt[:], vt[:])

                # ================= attention =================
                ew = [singles.tile([128, NT, T], FP8, tag=f"ew{h}",
                                   name=f"ew{h}{sfx}") for h in range(H)]
                a4 = singles.tile([128, NCHUNK, T], FP8, tag="a4",
                                  name=f"a4{sfx}")
                pa_t = {}   # head -> psum tile [65, T]
                exp_ctr = [0]

                def emit_qk_step(h, st):
                    # QK^T for head h, s-tile st -> pw psum, then exp -> ew
                    p, slot = h // 2, h % 2
                    lo, hi = CH * slot, CH * (slot + 1)
                    ktile, qtile = qk[2 * p], qk[2 * p + 1]
                    pw = pA.tile([128, T], F32, tag="pA", name="pw")
                    for nb in range(2):
                        sl = slice(512 * nb, 512 * (nb + 1))
                        nc.tensor.matmul(
                            pw[:, sl], ktile[lo:hi, 128 * st:128 * (st + 1)],
                            qtile[lo:hi, sl], start=True, stop=True)
                    i = exp_ctr[0]
                    exp_ctr[0] += 1
                    if (i * act_share) % 32 < act_share:
                        nc.scalar.activation(
                            ew[h][:, st, :], pw[:],
                            mybir.ActivationFunctionType.Exp, scale=ALPHA)
                    else:
                        nc.vector.tensor_scalar(
                            out=ew[h][:, st, :].bitcast(I8), in0=pw[:],
                            scalar1=EXP_A, scalar2=EXP_B,
                            op0=mybir.AluOpType.mult,
                            op1=mybir.AluOpType.add)

                def av_duty(h):
                    # AV matmuls for head h (consumes ew[h]); rider col 64
                    # of vt gives the softmax denominator in pa row 64.
                    pa = pPair.tile([128, T], F32, tag="pPair", name="pa")
                    pa_t[h] = pa
                    for u in range(4):
                        for ch4 in range(4):
                            sl = slice(256 * ch4, 256 * (ch4 + 1))
                            nc.tensor.matmul(
                                pa[:, sl],
                                vt[:, 2 * u:2 * u + 2, h, 0:128],
                                ew[h][:, 2 * u:2 * u + 2, sl],
                                start=(u == 0 and ch4 % 2 == 0),
                                stop=(u == 3), perf_mode=DR,
                                skip_group_check=True)
                            yield

                def emit_normalize(h, split=False):
                    p, slot = h // 2, h % 2
                    pa = pa_t[h]
                    if split:
                        rechs, rbhs = [], []
                        for nb in range(2):
                            sl = slice(512 * nb, 512 * (nb + 1))
                            rech = recp.tile([1, 512], F32, tag="rech",
                                             name="rech")
                            nc.vector.reciprocal_approx_fast(rech[:],
                                                             pa[0:1, sl])
                            rechs.append(rech)
                        for nb in range(2):
                            rbh = rbp.tile([CH, 512], F32, tag="rbh",
                                           name="rbh")
                            nc.gpsimd.partition_broadcast(rbh[:], rechs[nb][:])
                            rbhs.append(rbh)
                        for nb in range(2):
                            sl = slice(512 * nb, 512 * (nb + 1))
                            nc.vector.tensor_mul(
                                a4[CH * slot:CH * (slot + 1), p, sl],
                                pa[64:64 + CH, sl], rbhs[nb][:])
                        return
                    if debug:
                        if h == 0:
                            nc.sync.dma_start(dbg_ew[:], ew[0][:])
                        pa_cp = small.tile([128, T], F32, tag="dbgpa",
                                           name="pa_cp")
                        nc.vector.tensor_copy(pa_cp[:], pa[:])
                        nc.sync.dma_start(dbg_pa[h], pa_cp[:])
                    rec = recp.tile([1, T], F32, tag="rec", name="rec")
                    nc.vector.reciprocal_approx_fast(rec[:], pa[0:1, :])
                    rb = rbp.tile([CH, T], F32, tag="rb", name="rb")
                    nc.gpsimd.partition_broadcast(rb[:], rec[:])
                    nc.vector.tensor_mul(a4[CH * slot:CH * (slot + 1), p, :],
                                         pa[64:64 + CH, :], rb[:])
                    if debug:
                        nc.sync.dma_start(dbg_rec[h:h + 1, :], rec[:])
                        nc.sync.dma_start(dbg_rb[h:h + 1, :], rb[0:1, :])

                duties = [(v_duty(), None), (qk_duty(), None)]
                for h in range(H):
                    last = h == H - 1
                    for st in range(NT):
                        emit_qk_step(h, st)
                        adv = 0
                        di = 0
                        while duties and adv < 6:
                            try:
                                next(duties[di][0])
                                adv += 1
                                di = (di + 1) % min(len(duties), 2)
                            except StopIteration:
                                fin = duties.pop(di)
                                if fin[1] is not None:
                                    emit_normalize(fin[1],
                                                   split=(fin[1] >= 6))
                                di = 0
                        if last and st % 2 == 1:
                            # drain own AV eagerly: pair u=(st-1)/2 exps done
                            if st == 1:
                                last_duty = av_duty(h)
                            for _ in range(4):
                                try:
                                    next(last_duty)
                                except StopIteration:
                                    break
                    if not last:
                        duties.append((av_duty(h), h))
                        if h == H - 2:
                            pass
                    else:
                        for _ in last_duty:
                            pass
                        emit_normalize(h, split=True)
                # drain remaining AV duties + normalizes
                for g, hh in duties:
                    for _ in g:
                        pass
                    emit_normalize(hh)

                if debug:
                    nc.sync.dma_start(dbg_a4[:], a4[:])
                # ====== tail: proj + residual + output ======
                # stage-0 (c-pairs 0,1 = heads 0-3) emitted for m 0..2 before
                # the last normalizes land; fills the PE dead zone
                po_t = {}

                def proj_stage0(m):
                    po = pA.tile([128, T], F32, tag="pA", name="po")
                    po_t[m] = po
                    for ch4 in range(4):
                        sl = slice(256 * ch4, 256 * (ch4 + 1))
                        nc.tensor.matmul(
                            po[:, sl], wproj_sb[:, m, 0],
                            a4[:, 0:2, sl],
                            start=(ch4 % 2 == 0), stop=False, perf_mode=DR,
                            skip_group_check=True)

                for m in range(3):
                    proj_stage0(m)
                for m in range(NCHUNK):
                    if m not in po_t:
                        proj_stage0(m)
                    po = po_t[m]
                    for ch4 in range(4):
                        sl = slice(256 * ch4, 256 * (ch4 + 1))
                        nc.tensor.matmul(
                            po[:, sl], wproj_sb[:, m, 1],
                            a4[:, 2:4, sl],
                            start=False, stop=False, perf_mode=DR,
                            skip_group_check=True)
                    for ch4 in range(4):
                        sl = slice(256 * ch4, 256 * (ch4 + 1))
                        nc.tensor.matmul(
                            po[:, sl], bproj_sb[:, m], onesdr[:],
                            start=False, stop=False, perf_mode=DR,
                            skip_group_check=True)
                    for nb in range(2):
                        sl = slice(512 * nb, 512 * (nb + 1))
                        nc.tensor.matmul(
                            po[:, sl], id64_sb[:], x_sb[m][:, sl],
                            start=False, stop=True,
                            skip_group_check=True)
                    ot = outp.tile([128, T], BF16, tag="out", name="ot")
                    nc.scalar.activation(ot[:], po[:],
                                         mybir.ActivationFunctionType.Copy,
                                         scale=1.0 / (SA * SP))
                    if rep == n_reps - 1:
                        q = [nc.sync, nc.gpsimd, nc.sync, nc.scalar][m]
                        q.dma_start(out_d[m], ot[:])

    nc.compile()
    return nc


def _get_program(n_reps=1):
    key = ("prog", n_reps)
    if key not in _CACHE:
        _CACHE[key] = _build_program(n_reps)
    return _CACHE[key]


def kernel(x, gn_w, gn_b, qkv_w, qkv_b, proj_w, proj_b, _n_reps=1):
    hw = _host_weights(gn_w, gn_b, qkv_w, qkv_b, proj_w, proj_b)
    xr = np.ascontiguousarray(
        np.asarray(x, np.float32).reshape(B, NCHUNK, 128, T)
    ).astype(ml_dtypes.bfloat16)
    nc = _get_program(_n_reps)
    in_maps = [dict(hw, x=xr[b]) for b in range(B)]
    res = run_bass_kernel_spmd(nc, in_maps, core_ids=list(range(B)))
    out = np.stack([np.asarray(res.results[b]["out"]).astype(np.float32)
                    for b in range(B)])
    return out.reshape(B, C, HS, WS)
